# revision 3
# baseline (speedup 1.0000x reference)
"""Bass/Tile TRN2 kernel for nn_Decoder_Transformer (B=2, S=1024, D=1024, H=16,
L=4, DFF=4096, 3 output heads) on 8 NeuronCores.

Sharding: sequence-parallel. Core c owns 256 contiguous tokens: batch b=c//4,
rows [(c%4)*256, (c%4+1)*256) of that batch. Weights are replicated. Per layer,
each core computes q/k/v for its own tokens, K^T and V are AllGathered across
all 8 cores (two 8-way AllGathers), and each core unpacks only its batch's
sections (tc.If on a per-core selector) before running causal attention for its
query rows. LayerNorm / residuals / FFN / output heads are fully token-local.
Output rows are gathered on the host.

Matmul operands are fp16 (1 cycle/row on PE vs 4 for fp32); PSUM accumulation
and all vector math (softmax, LayerNorm, residuals) are fp32.
"""

import sys
import os

for _p in ("/opt/trn_rl_repo",):
    if _p not in sys.path and os.path.isdir(_p):
        sys.path.insert(0, _p)

import numpy as np

import concourse.bass as bass
import concourse.mybir as mybir
import concourse.tile as tile
from concourse import bacc
from concourse.bass_utils import run_bass_kernel_spmd
from concourse.masks import make_identity

F32 = mybir.dt.float32
AF = mybir.ActivationFunctionType
OP = mybir.AluOpType

# ---- problem constants -----------------------------------------------------
B, S, D, H, L, DFF = 2, 1024, 1024, 16, 4, 4096
DK = D // H            # 64
NOUT = 3
NC = 8                 # cores
T = 256                # tokens per core
TH = 2                 # 128-row tiles per core
DT = 8                 # D / 128
FT = DFF // 128        # 32
KB = 8                 # 128-token kv blocks per batch
OG = 2                 # 512-wide output column groups per 1024
LN_EPS = 1e-5

_CACHE = {}


def _build(dt_mm):
    nc = bacc.Bacc("TRN2", target_bir_lowering=False, debug=False,
                   enable_asserts=False, num_devices=NC)

    def din(name, shape, dt=dt_mm):
        return nc.dram_tensor(name, shape, dt, kind="ExternalInput").ap()

    # per-core inputs
    src = din("src", [128, TH], F32)
    pe = din("pe", [128, TH, D], F32)           # pe slice + emb_b, fp32
    embw = din("embw", [1, D], F32)
    sel = din("sel", [1, 1], mybir.dt.uint32)   # batch id (0/1)
    masks = din("masks", [128, KB, T])          # 0/1 causal masks, dt_mm
    # replicated weights (dt_mm)
    Wq = din("Wq", [L, D, D])
    Wk = din("Wk", [L, D, D])
    Wv = din("Wv", [L, D, D])
    Wo = din("Wo", [L, D, D])
    fc1w = din("fc1w", [L, D, DFF])
    fc2w = din("fc2w", [L, DFF, D])
    hw1 = din("hw1", [NOUT, D, D])
    hw2 = din("hw2", [128, NOUT, DT], F32)      # hw2[o, ft*128+p, 0] -> [p, o, ft]
    out = nc.dram_tensor("y", [T, NOUT], F32, kind="ExternalOutput").ap()

    with tile.TileContext(nc) as tc:
        with (
            tc.tile_pool(name="persist", bufs=1) as pers,
            tc.tile_pool(name="xpool", bufs=2) as xpool,
            tc.tile_pool(name="hot", bufs=2) as hot,        # y / attnfull / ff fp32 [128,TH,D]
            tc.tile_pool(name="ex", bufs=4) as exp_pool,
            tc.tile_pool(name="wpan", bufs=3) as wpan,      # [128, DT, 128] panels
            tc.tile_pool(name="wbig", bufs=2) as wbig,      # [128, DT, 512] panels
            tc.tile_pool(name="wblk", bufs=4) as wblk,      # [128, 512] blocks
            tc.tile_pool(name="small", bufs=4) as small,
            tc.tile_pool(name="psc", bufs=2, space="PSUM") as psc,
            tc.tile_pool(name="ppv", bufs=2, space="PSUM") as ppv,
            tc.tile_pool(name="pmm", bufs=2, space="PSUM") as pmm,
            tc.tile_pool(name="ptp", bufs=2, space="PSUM") as ptp,
            tc.tile_pool(name="dram", bufs=1, space="DRAM") as dram,
        ):
            # ---- persistent tiles ----
            ident = pers.tile([128, 128], F32)
            make_identity(nc, ident[:])
            src_sb = pers.tile([128, TH], F32)
            nc.sync.dma_start(src_sb[:], src[:])
            embw_sb = pers.tile([1, D], F32)
            nc.sync.dma_start(embw_sb[:], embw[:])
            embw_bc = pers.tile([128, D], F32)
            nc.gpsimd.partition_broadcast(embw_bc[:], embw_sb[:])
            sel_sb = pers.tile([1, 1], mybir.dt.uint32)
            nc.sync.dma_start(sel_sb[:], sel[:])
            mask_sb = pers.tile([128, KB, T], dt_mm)
            nc.sync.dma_start(mask_sb[:], masks[:])
            hw2_sb = pers.tile([128, NOUT, DT], F32)
            nc.sync.dma_start(hw2_sb[:], hw2[:])

            kT_full = pers.tile([128, DT, 1024], dt_mm)     # [d%128, d//128, kv tok]
            v_ext = pers.tile([128, KB, H * 65], dt_mm)     # per head: 64 v dims + ones col
            v_ext_r = v_ext[:].rearrange("p k (h e) -> p k h e", e=65)
            nc.vector.memset(v_ext_r[:, :, :, 64:65], 1.0)

            qT = pers.tile([128, DT, T], dt_mm)
            attnT = pers.tile([128, DT, T], dt_mm)
            xT = pers.tile([128, DT, T], dt_mm)
            ff1T = pers.tile([128, FT, T], dt_mm)

            # dram scratch for collectives (per layer: Shared outputs must have
            # a single writer)
            ag_k_ins = [dram.tile([D, T], dt_mm, tag=f"agki{i}", name=f"agki{i}")
                        for i in range(L)]
            ag_k_outs = [dram.tile([NC * D, T], dt_mm, addr_space="Shared",
                                   tag=f"agko{i}", name=f"agko{i}")
                         for i in range(L)]
            ag_v_ins = [dram.tile([T, D], dt_mm, tag=f"agvi{i}", name=f"agvi{i}")
                        for i in range(L)]
            ag_v_outs = [dram.tile([NC * T, D], dt_mm, addr_space="Shared",
                                   tag=f"agvo{i}", name=f"agvo{i}")
                         for i in range(L)]

            # ---- embedding: x = src*emb_w + (pe + emb_b) ----
            x = xpool.tile([128, TH, D], F32, tag="x")
            pe_sb = hot.tile([128, TH, D], F32, tag="hot")
            nc.sync.dma_start(pe_sb[:], pe[:])
            for th in range(TH):
                nc.vector.scalar_tensor_tensor(
                    x[:, th, :], embw_bc[:], src_sb[:, th:th + 1], pe_sb[:, th, :],
                    OP.mult, OP.add)

            def transpose_to(dst, src_x):
                # src_x fp32 [128, TH, D] -> dst dt_mm [128, DT, T] (xT layout)
                for th in range(TH):
                    for dt_i in range(DT):
                        tp = ptp.tile([128, 128], F32, tag="tp")
                        nc.tensor.transpose(
                            tp[:], src_x[:, th, dt_i * 128:(dt_i + 1) * 128], ident[:])
                        nc.scalar.copy(
                            dst[:, dt_i, th * 128:(th + 1) * 128], tp[:])

            def ln_inplace(y_t, resid, x_new):
                # x_new = LN(y_t) + resid   (gamma=1, beta=0)
                for th in range(TH):
                    st = small.tile([128, 2, 6], F32, tag="st")
                    nc.vector.bn_stats(st[:, 0, :], y_t[:, th, 0:512])
                    nc.vector.bn_stats(st[:, 1, :], y_t[:, th, 512:1024])
                    ag = small.tile([128, 2], F32, tag="ag")
                    nc.vector.bn_aggr(ag[:], st[:])
                    veps = small.tile([128, 1], F32, tag="veps")
                    nc.vector.tensor_scalar_add(veps[:], ag[:, 1:2], LN_EPS)
                    sd = small.tile([128, 1], F32, tag="sd")
                    nc.scalar.sqrt(sd[:], veps[:])
                    rstd = small.tile([128, 1], F32, tag="rstd")
                    nc.vector.reciprocal(rstd[:], sd[:])
                    xh = small.tile([128, D], F32, tag="xh")
                    nc.vector.tensor_scalar(
                        xh[:], y_t[:, th, :], ag[:, 0:1], rstd[:],
                        OP.subtract, OP.mult)
                    nc.vector.tensor_add(x_new[:, th, :], xh[:], resid[:, th, :])

            for l in range(L):
                ag_k_in, ag_k_out = ag_k_ins[l], ag_k_outs[l]
                ag_v_in, ag_v_out = ag_v_ins[l], ag_v_outs[l]
                with nc.named_scope(f"L{l}_qkv"):
                    transpose_to(xT, x)

                    # kT[dq, t] = sum_k Wk[k, dq] * xT[k, t]
                    for dq in range(DT):
                        pan = wpan.tile([128, DT, 128], dt_mm, tag="wpan")
                        nc.sync.dma_start(
                            pan[:],
                            Wk[l].rearrange("(kt p) m -> p kt m", p=128)
                            [:, :, dq * 128:(dq + 1) * 128])
                        pmk = pmm.tile([128, 512], F32, tag="mm")
                        for kt in range(DT):
                            nc.tensor.matmul(
                                pmk[:, 0:T], pan[:, kt, :], xT[:, kt, :],
                                start=(kt == 0), stop=(kt == DT - 1))
                        kts = small.tile([128, T], dt_mm, tag="kts")
                        nc.scalar.copy(kts[:], pmk[:, 0:T])
                        nc.sync.dma_start(
                            ag_k_in[dq * 128:(dq + 1) * 128, :], kts[:])
                    nc.gpsimd.collective_compute(
                        "AllGather", OP.bypass, replica_groups=[list(range(NC))],
                        ins=[ag_k_in.opt()], outs=[ag_k_out.opt()])

                    # v[t, dv] = sum_k xT[k, t] * Wv[k, dv]
                    for og in range(OG):
                        pan = wbig.tile([128, DT, 512], dt_mm, tag="wbig")
                        nc.sync.dma_start(
                            pan[:],
                            Wv[l].rearrange("(kt p) n -> p kt n", p=128)
                            [:, :, og * 512:(og + 1) * 512])
                        for th in range(TH):
                            pmv = pmm.tile([128, 512], F32, tag="mm")
                            for kt in range(DT):
                                nc.tensor.matmul(
                                    pmv[:], xT[:, kt, th * 128:(th + 1) * 128],
                                    pan[:, kt, :],
                                    start=(kt == 0), stop=(kt == DT - 1))
                            vts = small.tile([128, 512], dt_mm, tag="vts")
                            nc.scalar.copy(vts[:], pmv[:])
                            nc.sync.dma_start(
                                ag_v_in.rearrange("(a b) d -> b a d", a=TH)
                                [:, th, og * 512:(og + 1) * 512], vts[:])
                    nc.gpsimd.collective_compute(
                        "AllGather", OP.bypass, replica_groups=[list(range(NC))],
                        ins=[ag_v_in.opt()], outs=[ag_v_out.opt()])

                    # qT
                    for dq in range(DT):
                        pan = wpan.tile([128, DT, 128], dt_mm, tag="wpan")
                        nc.sync.dma_start(
                            pan[:],
                            Wq[l].rearrange("(kt p) m -> p kt m", p=128)
                            [:, :, dq * 128:(dq + 1) * 128])
                        pmq = pmm.tile([128, 512], F32, tag="mm")
                        for kt in range(DT):
                            nc.tensor.matmul(
                                pmq[:, 0:T], pan[:, kt, :], xT[:, kt, :],
                                start=(kt == 0), stop=(kt == DT - 1))
                        nc.scalar.copy(qT[:, dq, :], pmq[:, 0:T])

                with nc.named_scope(f"L{l}_unpack"):
                    # unpack my batch's 4 sections of K^T and V
                    rv = nc.sync.value_load(sel_sb[0:1, 0:1])
                    with tc.If(rv < 1) as cmp:
                        for j in range(4):
                            sec = j
                            nc.sync.dma_start(
                                kT_full[:, :, j * 256:(j + 1) * 256],
                                ag_k_out[sec * D:(sec + 1) * D, :]
                                .rearrange("(dt p) t -> p dt t", p=128))
                            for st_i in range(2):
                                nc.sync.dma_start(
                                    v_ext_r[:, 2 * j + st_i, :, 0:64],
                                    ag_v_out[sec * T + st_i * 128:
                                             sec * T + (st_i + 1) * 128, :]
                                    .rearrange("p (h e) -> p h e", e=64))
                    with cmp.Else():
                        for j in range(4):
                            sec = 4 + j
                            nc.sync.dma_start(
                                kT_full[:, :, j * 256:(j + 1) * 256],
                                ag_k_out[sec * D:(sec + 1) * D, :]
                                .rearrange("(dt p) t -> p dt t", p=128))
                            for st_i in range(2):
                                nc.sync.dma_start(
                                    v_ext_r[:, 2 * j + st_i, :, 0:64],
                                    ag_v_out[sec * T + st_i * 128:
                                             sec * T + (st_i + 1) * 128, :]
                                    .rearrange("p (h e) -> p h e", e=64))

                with nc.named_scope(f"L{l}_attn"):
                    for h in range(H):
                        hq, hd = (h % 2) * 64, h // 2
                        pv = ppv.tile([128, T], F32, tag="pv")
                        for kb in range(KB):
                            sc = psc.tile([128, T], F32, tag="sc")
                            nc.tensor.matmul(
                                sc[:], kT_full[hq:hq + 64, hd, kb * 128:(kb + 1) * 128],
                                qT[hq:hq + 64, hd, :], start=True, stop=True)
                            ex = exp_pool.tile([128, T], dt_mm, tag="ex")
                            nc.scalar.activation(ex[:], sc[:], AF.Exp, scale=0.125)
                            nc.vector.tensor_mul(ex[:], ex[:], mask_sb[:, kb, :])
                            nc.tensor.matmul(
                                pv[0:65, :], v_ext_r[:, kb, h, :], ex[:],
                                start=(kb == 0), stop=(kb == KB - 1),
                                skip_group_check=True)
                        den = small.tile([1, T], F32, tag="den")
                        nc.vector.tensor_scalar_add(den[:], pv[64:65, :], 1e-9)
                        rcp = small.tile([1, T], F32, tag="rcp")
                        nc.vector.reciprocal(rcp[:], den[:])
                        rb = small.tile([128, T], F32, tag="rb")
                        nc.gpsimd.partition_broadcast(rb[:], rcp[:])
                        nc.vector.tensor_tensor(
                            attnT[hq:hq + 64, hd, :], pv[0:64, :],
                            rb[hq:hq + 64, :], OP.mult)

                with nc.named_scope(f"L{l}_wo_ln1"):
                    attnfull = hot.tile([128, TH, D], F32, tag="hot")
                    for og in range(OG):
                        pan = wbig.tile([128, DT, 512], dt_mm, tag="wbig")
                        nc.sync.dma_start(
                            pan[:],
                            Wo[l].rearrange("(kt p) n -> p kt n", p=128)
                            [:, :, og * 512:(og + 1) * 512])
                        for th in range(TH):
                            pmo = pmm.tile([128, 512], F32, tag="mm")
                            for kt in range(DT):
                                nc.tensor.matmul(
                                    pmo[:], attnT[:, kt, th * 128:(th + 1) * 128],
                                    pan[:, kt, :],
                                    start=(kt == 0), stop=(kt == DT - 1))
                            nc.scalar.copy(
                                attnfull[:, th, og * 512:(og + 1) * 512], pmo[:])
                    y_t = hot.tile([128, TH, D], F32, tag="hot2")
                    for th in range(TH):
                        nc.vector.tensor_add(
                            y_t[:, th, :], x[:, th, :], attnfull[:, th, :])
                    x = xpool.tile([128, TH, D], F32, tag="x")
                    ln_inplace(y_t, attnfull, x)

                with nc.named_scope(f"L{l}_ffn"):
                    transpose_to(xT, x)
                    # ff1T[f, t] = relu(sum_k fc1w[k, f] * xT[k, t])
                    for ft in range(FT):
                        pan = wpan.tile([128, DT, 128], dt_mm, tag="wpan")
                        nc.sync.dma_start(
                            pan[:],
                            fc1w[l].rearrange("(kt p) m -> p kt m", p=128)
                            [:, :, ft * 128:(ft + 1) * 128])
                        pmf = pmm.tile([128, 512], F32, tag="mm")
                        for kt in range(DT):
                            nc.tensor.matmul(
                                pmf[:, 0:T], pan[:, kt, :], xT[:, kt, :],
                                start=(kt == 0), stop=(kt == DT - 1))
                        nc.scalar.activation(
                            ff1T[:, ft, :], pmf[:, 0:T], AF.Relu)
                    # ff = relu(ff1 @ fc2w)
                    ff = hot.tile([128, TH, D], F32, tag="hot")
                    for og in range(OG):
                        for th in range(TH):
                            pmf2 = pmm.tile([128, 512], F32, tag="mm")
                            for dft in range(FT):
                                blk = wblk.tile([128, 512], dt_mm, tag="wblk")
                                nc.sync.dma_start(
                                    blk[:],
                                    fc2w[l, dft * 128:(dft + 1) * 128,
                                         og * 512:(og + 1) * 512])
                                nc.tensor.matmul(
                                    pmf2[:], ff1T[:, dft, th * 128:(th + 1) * 128],
                                    blk[:],
                                    start=(dft == 0), stop=(dft == FT - 1))
                            nc.vector.tensor_scalar_max(
                                ff[:, th, og * 512:(og + 1) * 512], pmf2[:], 0.0)
                    y2 = hot.tile([128, TH, D], F32, tag="hot2")
                    for th in range(TH):
                        nc.vector.tensor_add(
                            y2[:, th, :], x[:, th, :], ff[:, th, :])
                    x = xpool.tile([128, TH, D], F32, tag="x")
                    ln_inplace(y2, ff, x)

            # ---- output heads ----
            with nc.named_scope("heads"):
                transpose_to(xT, x)
                out_sb = pers.tile([128, TH, NOUT], F32)
                for o in range(NOUT):
                    for ft in range(DT):
                        pan = wpan.tile([128, DT, 128], dt_mm, tag="wpan")
                        nc.sync.dma_start(
                            pan[:],
                            hw1[o].rearrange("(kt p) m -> p kt m", p=128)
                            [:, :, ft * 128:(ft + 1) * 128])
                        pmh = pmm.tile([128, 512], F32, tag="mm")
                        for kt in range(DT):
                            nc.tensor.matmul(
                                pmh[:, 0:T], pan[:, kt, :], xT[:, kt, :],
                                start=(kt == 0), stop=(kt == DT - 1))
                        nc.scalar.activation(
                            ff1T[:, ft, :], pmh[:, 0:T], AF.Relu)
                    # hw2 rhs is fp32; cast to dt_mm for the matmul
                    w2 = small.tile([128, DT], dt_mm, tag="w2")
                    nc.vector.tensor_copy(w2[:], hw2_sb[:, o, :])
                    for th in range(TH):
                        pho = ptp.tile([128, 128], F32, tag="tp")
                        for ft in range(DT):
                            nc.tensor.matmul(
                                pho[:, 0:1], ff1T[:, ft, th * 128:(th + 1) * 128],
                                w2[:, ft:ft + 1],
                                start=(ft == 0), stop=(ft == DT - 1))
                        nc.vector.tensor_copy(out_sb[:, th, o:o + 1], pho[:, 0:1])
                nc.sync.dma_start(
                    out[:].rearrange("(a b) o -> b a o", a=TH), out_sb[:])

    nc.compile()
    return nc


def _prep_inputs(inputs, dt_np):
    """Build the 8 per-core input maps from the full-problem inputs."""
    as_np = {k: np.asarray(v) for k, v in inputs.items()}
    g = as_np

    # specialization guard: biases / LN affine params are identity in this
    # problem (spec fills); the device program omits them.
    for name in ("bq", "bk", "bv", "bo", "fc1_b", "fc2_b", "hb1", "hb2",
                 "emb_b", "ln1_b", "ln2_b"):
        assert not np.any(g[name]), f"{name} must be zero for this kernel"
    for name in ("ln1_g", "ln2_g"):
        assert np.all(g[name] == 1.0), f"{name} must be ones for this kernel"

    wq = g["Wq"].astype(dt_np)
    wk = g["Wk"].astype(dt_np)
    wv = g["Wv"].astype(dt_np)
    wo = g["Wo"].astype(dt_np)
    fc1 = g["fc1_w"].astype(dt_np)
    fc2 = g["fc2_w"].astype(dt_np)
    hw1 = g["hw1"].astype(dt_np)
    hw2 = np.transpose(g["hw2"][:, :, 0].reshape(NOUT, DT, 128), (2, 0, 1))
    hw2 = np.ascontiguousarray(hw2, dtype=np.float32)
    embw = g["emb_w"].astype(np.float32)
    pe_full = g["pe"].astype(np.float32) + g["emb_b"][None, :].astype(np.float32)

    in_maps = []
    for c in range(NC):
        b, p = c // 4, c % 4
        rows = slice(p * T, (p + 1) * T)
        src_c = g["src"][b, rows, 0].astype(np.float32)        # [256]
        src_sb = np.ascontiguousarray(src_c.reshape(TH, 128).T)  # [128, TH]
        pe_c = pe_full[rows]                                    # [256, 1024]
        pe_sb = np.ascontiguousarray(
            np.transpose(pe_c.reshape(TH, 128, D), (1, 0, 2)))
        # causal masks: scoresT[kj_p, t]: valid iff kb*128 + kj_p <= p*256 + t
        kj = (np.arange(KB)[:, None, None] * 128 + np.arange(128)[None, :, None])
        qg = p * T + np.arange(T)[None, None, :]
        m = (kj <= qg).astype(dt_np)                            # [KB, 128, 256]
        m_sb = np.ascontiguousarray(np.transpose(m, (1, 0, 2)))  # [128, KB, 256]
        in_maps.append({
            "src": src_sb, "pe": pe_sb, "embw": embw,
            "sel": np.array([[b]], dtype=np.uint32),
            "masks": m_sb,
            "Wq": wq, "Wk": wk, "Wv": wv, "Wo": wo,
            "fc1w": fc1, "fc2w": fc2, "hw1": hw1, "hw2": hw2,
        })
    return in_maps


def kernel(**inputs) -> np.ndarray:
    dt_mm = mybir.dt.float16
    dt_np = np.float16
    key = ("prog", str(dt_mm))
    if key not in _CACHE:
        _CACHE[key] = _build(dt_mm)
    nc = _CACHE[key]
    in_maps = _prep_inputs(inputs, dt_np)
    res = run_bass_kernel_spmd(nc, in_maps, core_ids=list(range(NC)))
    full = np.zeros((B, S, NOUT), dtype=np.float32)
    for c in range(NC):
        b, p = c // 4, c % 4
        full[b, p * T:(p + 1) * T, :] = res.results[c]["y"]
    return full


if __name__ == "__main__":
    sys.path.insert(0, os.path.dirname(os.path.abspath(__file__)))
    import reference
    ins = reference.setup_inputs()
    want = np.asarray(reference.reference(**ins))
    got = kernel(**{k: np.asarray(v) for k, v in ins.items()})
    err = np.abs(got - want).max() / np.abs(want).max()
    print("Relative error:", err)
